# revision 7
# baseline (speedup 1.0000x reference)
import sys

sys.path.insert(0, "/opt/trn_rl_repo")
import numpy as np
import concourse.bass as bass
import concourse.bacc as bacc
import concourse.mybir as mybir
from concourse.tile import TileContext
from concourse.bass import ts

F32 = mybir.dt.float32
F16 = mybir.dt.float16

# Problem dims (hardcoded per spec)
N, D, H, W, C = 2, 32, 32, 32, 128
G, Cg, K = 8, 16, 27
OM = 864  # G * K * 4
L = D * H * W

# Sharding: 8 cores = (n in 2) x (d-block in 4); 8 own d-slices per core.
NCORES = 8
DB = 8          # own d slices per core
DL = DB + 4     # slab depth with halo 2 on each side
HP, WP = H + 4, W + 4   # padded h/w (halo 2) for shift addressing
SLAB = DL * H * W        # 12288 input slab voxels
VOWN = DB * H * W        # 8192 own voxels
PADVOL = DL * HP * WP    # padded v volume free size

_NC_CACHE = {}


def _perm_rows():
    """Permute offmask rows: per group [od(27), oh(27), ow(27), m(27)]."""
    perm = np.zeros(OM, dtype=np.int64)
    p = 0
    for g in range(G):
        base = g * 108
        for axis in range(3):
            for k in range(K):
                perm[p] = base + k * 3 + axis
                p += 1
        for k in range(K):
            perm[p] = base + 81 + k
            p += 1
    return perm


def build_nc():
    nc = bacc.Bacc("TRN2", target_bir_lowering=False, debug=False, num_devices=8)
    xslab = nc.declare_dram_parameter("xslab", [SLAB, C], F32, isOutput=False)
    vwT = nc.declare_dram_parameter("vwT", [C, C], F32, isOutput=False)
    vb = nc.declare_dram_parameter("vb", [C, 1], F32, isOutput=False)
    omT = nc.declare_dram_parameter("omT", [C, OM], F32, isOutput=False)
    ombr = nc.declare_dram_parameter("ombr", [128, OM], F32, isOutput=False)
    owT = nc.declare_dram_parameter("owT", [C, C], F16, isOutput=False)
    ob = nc.declare_dram_parameter("ob", [C, 1], F32, isOutput=False)
    idin = nc.declare_dram_parameter("idin", [128, 128], F32, isOutput=False)
    out_d = nc.declare_dram_parameter("out", [VOWN, C], F32, isOutput=True)

    with TileContext(nc) as tc:
        with (
            tc.tile_pool(name="const", bufs=1) as cpool,
            tc.tile_pool(name="big", bufs=1) as bigpool,
            tc.tile_pool(name="xio", bufs=3) as xpool,
            tc.tile_pool(name="ps_tr", bufs=3, space="PSUM") as ps_tr,
            tc.tile_pool(name="ps_mm", bufs=2, space="PSUM") as ps_mm,
            tc.tile_pool(name="ps_om", bufs=1, space="PSUM") as ps_om,
            tc.tile_pool(name="sbuild", bufs=1) as spool,
            tc.tile_pool(name="stp", bufs=2) as stpool,
            tc.tile_pool(name="srep", bufs=4) as reppool,
            tc.tile_pool(name="accp", bufs=2) as accpool,
            tc.tile_pool(name="outp", bufs=3) as opool,
        ):
            ident = cpool.tile([128, 128], F32)
            nc.sync.dma_start(out=ident[:], in_=idin[:])
            vw_sb = cpool.tile([C, C], F32)
            nc.sync.dma_start(out=vw_sb[:], in_=vwT[:])
            om_w = cpool.tile([C, OM], F32)
            nc.sync.dma_start(out=om_w[:], in_=omT[:])
            ombr_sb = cpool.tile([128, OM], F32)
            nc.sync.dma_start(out=ombr_sb[:], in_=ombr[:])
            ow_sb = cpool.tile([C, C], F16)
            nc.sync.dma_start(out=ow_sb[:], in_=owT[:])
            vb_sb = cpool.tile([C, 1], F32)
            nc.sync.dma_start(out=vb_sb[:], in_=vb[:])
            ob_sb = cpool.tile([C, 1], F32)
            nc.sync.dma_start(out=ob_sb[:], in_=ob[:])
            cone = cpool.tile([128, 1], F32)
            nc.gpsimd.memset(cone[:], 1.0)
            cneg = cpool.tile([128, 1], F32)
            nc.gpsimd.memset(cneg[:], -1.0)

            # ---- Stage A: xT [ci, SLAB] via PE transpose ----
            xT = bigpool.tile([C, SLAB], F32)
            for t in range(SLAB // 128):
                xt_in = xpool.tile([128, 128], F32, tag="xin")
                nc.sync.dma_start(out=xt_in[:], in_=xslab[ts(t, 128), :])
                ps = ps_tr.tile([128, 128], F32, tag="tr")
                nc.tensor.transpose(out=ps[:], in_=xt_in[:], identity=ident[:])
                nc.scalar.copy(out=xT[:, ts(t, 128)], in_=ps[:])

            # ---- Stage B: value proj -> v_pad f16 [c, PADVOL] ----
            v_pad = bigpool.tile([C, PADVOL], F16)
            nc.gpsimd.memset(v_pad[:], 0.0)
            for t in range(SLAB // 512):
                ps = ps_mm.tile([128, 512], F32, tag="mm")
                nc.tensor.matmul(ps[:], vw_sb[:], xT[:, ts(t, 512)], start=True, stop=True)
                d_loc, h0 = t // 2, (t % 2) * 16
                off = (d_loc * HP + h0 + 2) * WP + 2
                dst = bass.AP(v_pad.tensor, v_pad.offset + off, [v_pad.ap[0], [WP, 16], [1, W]])
                src = bass.AP(ps.tensor, ps.offset, [ps.ap[0], [32, 16], [1, 32]])
                nc.scalar.activation(out=dst, in_=src, func=mybir.ActivationFunctionType.Identity, bias=vb_sb[:], scale=1.0)

            # per-slice loop
            for s in range(DB):
                # ---- Stage C: offset/mask proj for this slice (8 blocks) ----
                # ---- Stage D: build stencil S [vox, g*128] f32 ----
                ST = stpool.tile([128, 8 * 1024], F16, tag="ST")  # 8 tiles [128,1024] concat
                for b in range(8):
                    blk = s * 8 + b
                    ps = ps_om.tile([128, 1024], F32, tag="om")
                    xsl = xT[:, 2048 + 128 * blk:2048 + 128 * blk + 128]
                    nc.tensor.matmul(ps[:, 0:432], xsl, om_w[:, 0:432], start=True, stop=True)
                    nc.tensor.matmul(ps[:, 512:944], xsl, om_w[:, 432:864], start=True, stop=True)
                    om_sb = spool.tile([128, OM], F32, tag="om_sb")
                    nc.vector.scalar_tensor_tensor(
                        out=om_sb[:, 0:432], in0=ps[:, 0:432], scalar=1.0,
                        in1=ombr_sb[:, 0:432], op0=mybir.AluOpType.mult, op1=mybir.AluOpType.add)
                    nc.vector.scalar_tensor_tensor(
                        out=om_sb[:, 432:864], in0=ps[:, 512:944], scalar=1.0,
                        in1=ombr_sb[:, 432:864], op0=mybir.AluOpType.mult, op1=mybir.AluOpType.add)

                    # W3 [3v,3a,8g,27k] (648,216,27,1)
                    W3 = spool.tile([128, 1944], F32, tag="W3")
                    tmp = spool.tile([128, 648], F32, tag="tmp")
                    po = om_sb.tensor
                    ooff = om_sb.offset
                    ap_off = lambda shape: bass.AP(po, ooff, [om_sb.ap[0]] + shape)
                    w3ap = lambda v, shape: bass.AP(W3.tensor, W3.offset + v * 648, [W3.ap[0]] + shape)
                    # in: om offsets [(g,108),(a,27),(k,1)]
                    in_offs = ap_off([[108, 8], [27, 3], [1, 27]])
                    out_gak = lambda v: bass.AP(W3.tensor, W3.offset + v * 648, [W3.ap[0], [27, 8], [216, 3], [1, 27]])
                    RELU = mybir.ActivationFunctionType.Relu
                    nc.scalar.activation(out=out_gak(2), in_=in_offs, func=RELU, scale=1.0)
                    nc.scalar.activation(out=out_gak(0), in_=in_offs, func=RELU, scale=cneg[:])
                    tmp_gak = bass.AP(tmp.tensor, tmp.offset, [tmp.ap[0], [81, 8], [27, 3], [1, 27]])
                    nc.scalar.activation(out=tmp_gak, in_=in_offs, func=mybir.ActivationFunctionType.Abs, scale=1.0)
                    nc.scalar.activation(out=out_gak(1), in_=tmp_gak, func=mybir.ActivationFunctionType.Identity, scale=cneg[:], bias=cone[:])

                    # MWW [3v,8g,27k] = W3[v,a=2,g,k] * m[g,k]
                    MWW = spool.tile([128, 648], F32, tag="MWW")
                    nc.vector.tensor_tensor(
                        out=bass.AP(MWW.tensor, MWW.offset, [MWW.ap[0], [216, 3], [27, 8], [1, 27]]),
                        in0=bass.AP(W3.tensor, W3.offset + 432, [W3.ap[0], [648, 3], [27, 8], [1, 27]]),
                        in1=bass.AP(po, ooff + 81, [om_sb.ap[0], [0, 3], [108, 8], [1, 27]]),
                        op=mybir.AluOpType.mult)
                    # P1 [3dd,3dh,8g,27k] = W3[dd,0,g,k]*W3[dh,1,g,k]
                    P1 = spool.tile([128, 1944], F32, tag="P1")
                    nc.vector.tensor_tensor(
                        out=bass.AP(P1.tensor, P1.offset, [P1.ap[0], [648, 3], [216, 3], [1, 216]]),
                        in0=bass.AP(W3.tensor, W3.offset, [W3.ap[0], [648, 3], [0, 3], [1, 216]]),
                        in1=bass.AP(W3.tensor, W3.offset + 216, [W3.ap[0], [0, 3], [648, 3], [1, 216]]),
                        op=mybir.AluOpType.mult)
                    # T3 [3dd,3dh,3dw,8g,27k]: 3 ops (per dw)
                    T3 = spool.tile([128, 5832], F32, tag="T3")
                    for dw in range(3):
                        nc.vector.tensor_tensor(
                            out=bass.AP(T3.tensor, T3.offset + dw * 216, [T3.ap[0], [1944, 3], [648, 3], [1, 216]]),
                            in0=bass.AP(P1.tensor, P1.offset, [P1.ap[0], [648, 3], [216, 3], [1, 216]]),
                            in1=bass.AP(MWW.tensor, MWW.offset + dw * 216, [MWW.ap[0], [0, 3], [0, 3], [1, 216]]),
                            op=mybir.AluOpType.mult)
                    # S [8g, 128] with Delta lin (Dd*5+Dh)*5+Dw in 0..124
                    S_sb = spool.tile([128, 1024], F32, tag="S_sb")
                    nc.gpsimd.memset(S_sb[:], 0.0)
                    for dd in range(3):
                        for dh in range(3):
                            for dw in range(3):
                                soff = (dd * 25 + dh * 5 + dw) * 8
                                t3off = (dd * 9 + dh * 3 + dw) * 216
                                for kd in range(3):
                                    sap = bass.AP(S_sb.tensor, S_sb.offset + soff + kd * 200,
                                                  [S_sb.ap[0], [40, 3], [8, 3], [1, 8]])
                                    nc.vector.tensor_tensor(
                                        out=sap, in0=sap,
                                        in1=bass.AP(T3.tensor, T3.offset + t3off + kd * 9,
                                                    [T3.ap[0], [3, 3], [1, 3], [27, 8]]),
                                        op=mybir.AluOpType.add)
                    # ---- Stage E: transpose S -> ST tiles [16d x 8g, vox] f16 ----
                    for t in range(8):
                        inap = bass.AP(S_sb.tensor, S_sb.offset + 128 * t, [S_sb.ap[0], [1, 128]])
                        ps2 = ps_tr.tile([128, 128], F32, tag="tr")
                        nc.tensor.matmul(ps2[:], inap, ident[:], is_transpose=True)
                        nc.scalar.copy(out=ST[:, t * 1024 + b * 128:t * 1024 + b * 128 + 128], in_=ps2[:])

                # ---- Stage F: apply 125 shifts ----
                acc = accpool.tile([128, 1024], F16, tag="acc")
                tmpa = accpool.tile([128, 1024], F16, tag="tmpa")
                for delta in range(125):
                    Dd, rem = delta // 25, delta % 25
                    Dh, Dw = rem // 5, rem % 5
                    t, dsub = delta // 16, delta % 16
                    srep = reppool.tile([128, 1024], F16, tag="srep")
                    src = bass.AP(ST.tensor, ST.offset + t * 1024 + dsub * 8 * ST.ap[0][0],
                                  [[ST.ap[0][0], 8], [0, 16], [1, 1024]])
                    nc.sync.dma_start(out=srep[:], in_=src)
                    voff = ((s + Dd) * HP + Dh) * WP + Dw
                    vap = bass.AP(v_pad.tensor, v_pad.offset + voff, [v_pad.ap[0], [WP, 32], [1, 32]])
                    dst = acc if delta == 0 else tmpa
                    nc.vector.tensor_tensor(
                        out=bass.AP(dst.tensor, dst.offset, [dst.ap[0], [32, 32], [1, 32]]),
                        in0=vap,
                        in1=bass.AP(srep.tensor, srep.offset, [srep.ap[0], [32, 32], [1, 32]]),
                        op=mybir.AluOpType.mult)
                    if delta > 0:
                        nc.vector.tensor_tensor(out=acc[:], in0=acc[:], in1=tmpa[:], op=mybir.AluOpType.add)

                # ---- Stage G: out proj + transpose + store ----
                yT = opool.tile([128, 1024], F32, tag="yT")
                for hh in range(2):
                    ps = ps_mm.tile([128, 512], F32, tag="mm")
                    nc.tensor.matmul(ps[:], ow_sb[:], acc[:, ts(hh, 512)], start=True, stop=True)
                    nc.scalar.activation(out=yT[:, ts(hh, 512)], in_=ps[:], func=mybir.ActivationFunctionType.Identity, bias=ob_sb[:], scale=1.0)
                for t in range(8):
                    ps2 = ps_tr.tile([128, 128], F32, tag="tr")
                    nc.tensor.matmul(ps2[:], yT[:, ts(t, 128)], ident[:], is_transpose=True)
                    ob_t = opool.tile([128, 128], F32, tag="ob_t")
                    nc.scalar.copy(out=ob_t[:], in_=ps2[:])
                    nc.sync.dma_start(out=out_d[ts(s * 8 + t, 128), :], in_=ob_t[:])
    nc.compile()
    return nc


def prep_inputs(input, value_w, value_b, offmask_w, offmask_b, out_w, out_b):
    perm = _perm_rows()
    vwT = np.ascontiguousarray(np.asarray(value_w, np.float32).T)
    vb = np.asarray(value_b, np.float32).reshape(C, 1)
    omT = np.ascontiguousarray(np.asarray(offmask_w, np.float32)[perm].T)
    ombr = np.tile(np.asarray(offmask_b, np.float32)[perm][None, :], (128, 1))
    owT = np.ascontiguousarray(np.asarray(out_w, np.float32).T.astype(np.float16))
    ob = np.asarray(out_b, np.float32).reshape(C, 1)
    ident = np.eye(128, dtype=np.float32)
    xf = np.asarray(input, np.float32).reshape(N, D, H * W * C)
    in_maps = []
    for core in range(NCORES):
        n, db = core // 4, core % 4
        d0 = db * DB
        slab = np.zeros((DL, H * W * C), np.float32)
        lo, hi = max(0, d0 - 2), min(D, d0 + DB + 2)
        slab[lo - (d0 - 2):hi - (d0 - 2)] = xf[n, lo:hi]
        in_maps.append(dict(xslab=np.ascontiguousarray(slab.reshape(SLAB, C)),
                            vwT=vwT, vb=vb, omT=omT, ombr=ombr, owT=owT, ob=ob, idin=ident))
    return in_maps


def kernel(**inputs):
    from concourse.bass_utils import run_bass_kernel_spmd
    if "nc" not in _NC_CACHE:
        _NC_CACHE["nc"] = build_nc()
    nc = _NC_CACHE["nc"]
    in_maps = prep_inputs(**inputs)
    res = run_bass_kernel_spmd(nc, in_maps, list(range(NCORES))).results
    out = np.zeros((N, D, H, W, C), np.float32)
    for core in range(NCORES):
        n, db = core // 4, core % 4
        out[n, db * DB:(db + 1) * DB] = res[core]["out"].reshape(DB, H, W, C)
    return out


# revision 12
# speedup vs baseline: 1388.8042x; 1388.8042x over previous
import sys

sys.path.insert(0, "/opt/trn_rl_repo")
import numpy as np
import concourse.bass as bass
import concourse.bacc as bacc
import concourse.mybir as mybir
from concourse.tile import TileContext
from concourse.bass import ts

F32 = mybir.dt.float32
F16 = mybir.dt.float16

# Problem dims (hardcoded per spec)
N, D, H, W, C = 2, 32, 32, 32, 128
G, Cg, K = 8, 16, 27
OM = 864  # G * K * 4
L = D * H * W

# Sharding: 8 cores = (n in 2) x (d-block in 4); 8 own d-slices per core.
NCORES = 8
DB = 8          # own d slices per core
DL = DB + 4     # slab depth with halo 2 on each side
HP, WP = H + 4, W + 4   # padded h/w (halo 2) for shift addressing
SLAB = DL * H * W        # 12288 input slab voxels
VOWN = DB * H * W        # 8192 own voxels
PADVOL = DL * HP * WP    # padded v volume free size

_NC_CACHE = {}


def _perm_rows():
    """Permute offmask rows: per group [od(27), oh(27), ow(27), m(27)]."""
    perm = np.zeros(OM, dtype=np.int64)
    p = 0
    for g in range(G):
        base = g * 108
        for axis in range(3):
            for k in range(K):
                perm[p] = base + k * 3 + axis
                p += 1
        for k in range(K):
            perm[p] = base + 81 + k
            p += 1
    return perm


def build_nc():
    nc = bacc.Bacc("TRN2", target_bir_lowering=False, debug=False, num_devices=8)
    xslab = nc.declare_dram_parameter("xslab", [SLAB, C], F32, isOutput=False)
    vwT = nc.declare_dram_parameter("vwT", [C, C], F32, isOutput=False)
    vb = nc.declare_dram_parameter("vb", [C, 1], F32, isOutput=False)
    omT = nc.declare_dram_parameter("omT", [C, OM], F32, isOutput=False)
    ombr = nc.declare_dram_parameter("ombr", [128, OM], F32, isOutput=False)
    owT = nc.declare_dram_parameter("owT", [C, C], F16, isOutput=False)
    ob = nc.declare_dram_parameter("ob", [C, 1], F32, isOutput=False)
    idin = nc.declare_dram_parameter("idin", [128, 128], F32, isOutput=False)
    out_d = nc.declare_dram_parameter("out", [VOWN, C], F32, isOutput=True)

    with TileContext(nc) as tc:
        with (
            tc.tile_pool(name="const", bufs=1) as cpool,
            tc.tile_pool(name="big", bufs=1) as bigpool,
            tc.tile_pool(name="xio", bufs=3) as xpool,
            tc.tile_pool(name="ps_tr", bufs=3, space="PSUM") as ps_tr,
            tc.tile_pool(name="ps_mm", bufs=2, space="PSUM") as ps_mm,
            tc.tile_pool(name="ps_om", bufs=1, space="PSUM") as ps_om,
            tc.tile_pool(name="sbuild", bufs=1) as spool,
            tc.tile_pool(name="stp", bufs=2) as stpool,
            tc.tile_pool(name="srep", bufs=4) as reppool,
            tc.tile_pool(name="accp", bufs=2) as accpool,
            tc.tile_pool(name="outp", bufs=3) as opool,
        ):
            ident = cpool.tile([128, 128], F32)
            nc.sync.dma_start(out=ident[:], in_=idin[:])
            vw_sb = cpool.tile([C, C], F32)
            nc.sync.dma_start(out=vw_sb[:], in_=vwT[:])
            om_w = cpool.tile([C, OM], F32)
            nc.sync.dma_start(out=om_w[:], in_=omT[:])
            ombr_sb = cpool.tile([128, OM], F32)
            nc.sync.dma_start(out=ombr_sb[:], in_=ombr[:])
            ow_sb = cpool.tile([C, C], F16)
            nc.sync.dma_start(out=ow_sb[:], in_=owT[:])
            vb_sb = cpool.tile([C, 1], F32)
            nc.sync.dma_start(out=vb_sb[:], in_=vb[:])
            ob_sb = cpool.tile([C, 1], F32)
            nc.sync.dma_start(out=ob_sb[:], in_=ob[:])
            cone = cpool.tile([128, 1], F32)
            nc.gpsimd.memset(cone[:], 1.0)
            cneg = cpool.tile([128, 1], F32)
            nc.gpsimd.memset(cneg[:], -1.0)
            ident16 = cpool.tile([128, 128], F16)
            nc.vector.tensor_copy(out=ident16[:], in_=ident[:])

            # ---- Stage A: xT [ci, SLAB] via PE transpose ----
            xT = bigpool.tile([C, SLAB], F32)
            for t in range(SLAB // 128):
                xt_in = xpool.tile([128, 128], F32, tag="xin")
                nc.sync.dma_start(out=xt_in[:], in_=xslab[ts(t, 128), :])
                ps = ps_tr.tile([128, 128], F32, tag="tr")
                nc.tensor.transpose(out=ps[:], in_=xt_in[:], identity=ident[:])
                nc.scalar.copy(out=xT[:, ts(t, 128)], in_=ps[:])

            # ---- Stage B: value proj -> v_pad f16 [c, PADVOL] ----
            v_pad = bigpool.tile([C, PADVOL], F16)
            nc.gpsimd.memset(v_pad[:], 0.0)
            for t in range(SLAB // 512):
                ps = ps_mm.tile([128, 512], F32, tag="mm")
                nc.tensor.matmul(ps[:], vw_sb[:], xT[:, ts(t, 512)], start=True, stop=True)
                d_loc, h0 = t // 2, (t % 2) * 16
                off = (d_loc * HP + h0 + 2) * WP + 2
                dst = bass.AP(v_pad.tensor, v_pad.offset + off, [v_pad.ap[0], [WP, 16], [1, W]])
                src = bass.AP(ps.tensor, ps.offset, [ps.ap[0], [32, 16], [1, 32]])
                nc.scalar.activation(out=dst, in_=src, func=mybir.ActivationFunctionType.Identity, bias=vb_sb[:], scale=1.0)

            # per-slice loop
            for s in range(DB):
                # ---- Stage C: offset/mask proj for this slice (8 blocks) ----
                # ---- Stage D: build stencil S [vox, g*128] f32 ----
                ST = stpool.tile([128, 8 * 1024], F16, tag="ST")  # 8 tiles [128,1024] concat
                for b in range(8):
                    blk = s * 8 + b
                    ps = ps_om.tile([128, 1024], F32, tag="om")
                    xsl = xT[:, 2048 + 128 * blk:2048 + 128 * blk + 128]
                    nc.tensor.matmul(ps[:, 0:432], xsl, om_w[:, 0:432], start=True, stop=True)
                    nc.tensor.matmul(ps[:, 512:944], xsl, om_w[:, 432:864], start=True, stop=True)
                    om_sb = spool.tile([128, OM], F32, tag="om_sb")
                    nc.vector.scalar_tensor_tensor(
                        out=om_sb[:, 0:432], in0=ps[:, 0:432], scalar=1.0,
                        in1=ombr_sb[:, 0:432], op0=mybir.AluOpType.mult, op1=mybir.AluOpType.add)
                    nc.vector.scalar_tensor_tensor(
                        out=om_sb[:, 432:864], in0=ps[:, 512:944], scalar=1.0,
                        in1=ombr_sb[:, 432:864], op0=mybir.AluOpType.mult, op1=mybir.AluOpType.add)

                    # W3 [3v,3a,8g,27k] (648,216,27,1)
                    W3 = spool.tile([128, 1944], F16, tag="W3")
                    tmp = spool.tile([128, 648], F16, tag="tmp")
                    po = om_sb.tensor
                    ooff = om_sb.offset
                    ap_off = lambda shape: bass.AP(po, ooff, [om_sb.ap[0]] + shape)
                    w3ap = lambda v, shape: bass.AP(W3.tensor, W3.offset + v * 648, [W3.ap[0]] + shape)
                    # in: om offsets [(g,108),(a,27),(k,1)]
                    in_offs = ap_off([[108, 8], [27, 3], [1, 27]])
                    out_gak = lambda v: bass.AP(W3.tensor, W3.offset + v * 648, [W3.ap[0], [1, 8], [216, 3], [8, 27]])
                    RELU = mybir.ActivationFunctionType.Relu
                    nc.scalar.activation(out=out_gak(2), in_=in_offs, func=RELU, scale=1.0)
                    nc.scalar.activation(out=out_gak(0), in_=in_offs, func=RELU, scale=cneg[:])
                    tmp_gak = bass.AP(tmp.tensor, tmp.offset, [tmp.ap[0], [1, 8], [216, 3], [8, 27]])
                    nc.scalar.activation(out=tmp_gak, in_=in_offs, func=mybir.ActivationFunctionType.Abs, scale=1.0)
                    nc.scalar.activation(out=out_gak(1), in_=tmp_gak, func=mybir.ActivationFunctionType.Identity, scale=cneg[:], bias=cone[:])

                    # masks -> f16 compact [8g,27k]
                    m16 = spool.tile([128, 216], F16, tag="m16")
                    nc.scalar.activation(
                        out=bass.AP(m16.tensor, m16.offset, [m16.ap[0], [1, 8], [8, 27]]),
                        in_=bass.AP(po, ooff + 81, [om_sb.ap[0], [108, 8], [1, 27]]),
                        func=mybir.ActivationFunctionType.Identity, scale=1.0)
                    # MWW [3v,8g,27k] = W3[v,a=2,g,k] * m[g,k]
                    MWW = spool.tile([128, 648], F16, tag="MWW")
                    nc.vector.tensor_tensor(
                        out=bass.AP(MWW.tensor, MWW.offset, [MWW.ap[0], [216, 3], [1, 216]]),
                        in0=bass.AP(W3.tensor, W3.offset + 432, [W3.ap[0], [648, 3], [1, 216]]),
                        in1=bass.AP(m16.tensor, m16.offset, [m16.ap[0], [0, 3], [1, 216]]),
                        op=mybir.AluOpType.mult)
                    # P1 [3dd,3dh,8g,27k] = W3[dd,0,g,k]*W3[dh,1,g,k]
                    P1 = spool.tile([128, 1944], F16, tag="P1")
                    nc.vector.tensor_tensor(
                        out=bass.AP(P1.tensor, P1.offset, [P1.ap[0], [648, 3], [216, 3], [1, 216]]),
                        in0=bass.AP(W3.tensor, W3.offset, [W3.ap[0], [648, 3], [0, 3], [1, 216]]),
                        in1=bass.AP(W3.tensor, W3.offset + 216, [W3.ap[0], [0, 3], [648, 3], [1, 216]]),
                        op=mybir.AluOpType.mult)
                    # T3 [3dd,3dh,3dw,8g,27k]: 3 ops (per dw)
                    T3 = spool.tile([128, 5832], F16, tag="T3")
                    for dw in range(3):
                        nc.vector.tensor_tensor(
                            out=bass.AP(T3.tensor, T3.offset + dw * 216, [T3.ap[0], [1944, 3], [648, 3], [1, 216]]),
                            in0=bass.AP(P1.tensor, P1.offset, [P1.ap[0], [648, 3], [216, 3], [1, 216]]),
                            in1=bass.AP(MWW.tensor, MWW.offset + dw * 216, [MWW.ap[0], [0, 3], [0, 3], [1, 216]]),
                            op=mybir.AluOpType.mult)
                    # S [8g, 128] with Delta lin (Dd*5+Dh)*5+Dw in 0..124
                    S_sb = spool.tile([128, 1024], F16, tag="S_sb")
                    nc.gpsimd.memset(S_sb[:], 0.0)
                    bin_eng = nc.vector
                    for dd in range(3):
                        for dh in range(3):
                            for dw in range(3):
                                soff = (dd * 25 + dh * 5 + dw) * 8
                                t3off = (dd * 9 + dh * 3 + dw) * 216
                                sap = bass.AP(S_sb.tensor, S_sb.offset + soff,
                                              [S_sb.ap[0], [200, 3], [40, 3], [8, 3], [1, 8]])
                                bin_eng.tensor_tensor(
                                    out=sap, in0=sap,
                                    in1=bass.AP(T3.tensor, T3.offset + t3off,
                                                [T3.ap[0], [72, 3], [24, 3], [8, 3], [1, 8]]),
                                    op=mybir.AluOpType.add)
                    # ---- Stage E: transpose S -> ST tiles [16d x 8g, vox] f16 ----
                    for t in range(8):
                        inap = bass.AP(S_sb.tensor, S_sb.offset + 128 * t, [S_sb.ap[0], [1, 128]])
                        ps2 = ps_tr.tile([128, 128], F16, tag="tr")
                        nc.tensor.matmul(ps2[:], inap, ident16[:], is_transpose=True)
                        nc.scalar.copy(out=ST[:, t * 1024 + b * 128:t * 1024 + b * 128 + 128], in_=ps2[:])

                # ---- Stage F: apply 125 shifts ----
                acc = accpool.tile([128, 1024], F16, tag="acc")
                tmpa = accpool.tile([128, 1024], F16, tag="tmpa")
                NGPD = 0
                acc_g = tmpg = None
                if NGPD:
                    acc_g = accpool.tile([128, 1024], F16, tag="acc_g")
                    tmpg = accpool.tile([128, 1024], F16, tag="tmpg")
                for delta in range(125):
                    Dd, rem = delta // 25, delta % 25
                    Dh, Dw = rem // 5, rem % 5
                    t, dsub = delta // 16, delta % 16
                    on_gp = (NGPD > 0) and (delta >= 125 - NGPD)
                    eng = nc.gpsimd if on_gp else nc.vector
                    first = (delta == 125 - NGPD) if on_gp else (delta == 0)
                    a_t, t_t = (acc_g, tmpg) if on_gp else (acc, tmpa)
                    srep = reppool.tile([128, 1024], F16, tag="srep")
                    src = bass.AP(ST.tensor, ST.offset + t * 1024 + dsub * 8 * ST.ap[0][0],
                                  [[ST.ap[0][0], 8], [0, 16], [1, 1024]])
                    nc.sync.dma_start(out=srep[:], in_=src)
                    voff = ((s + Dd) * HP + Dh) * WP + Dw
                    vap = bass.AP(v_pad.tensor, v_pad.offset + voff, [v_pad.ap[0], [WP, 32], [1, 32]])
                    dst = a_t if first else t_t
                    eng.tensor_tensor(
                        out=bass.AP(dst.tensor, dst.offset, [dst.ap[0], [32, 32], [1, 32]]),
                        in0=vap,
                        in1=bass.AP(srep.tensor, srep.offset, [srep.ap[0], [32, 32], [1, 32]]),
                        op=mybir.AluOpType.mult)
                    if not first:
                        eng.tensor_tensor(out=a_t[:], in0=a_t[:], in1=t_t[:], op=mybir.AluOpType.add)
                if NGPD > 0:
                    nc.vector.tensor_tensor(out=acc[:], in0=acc[:], in1=acc_g[:], op=mybir.AluOpType.add)

                # ---- Stage G: out proj + transpose + store ----
                yT = opool.tile([128, 1024], F32, tag="yT")
                for hh in range(2):
                    ps = ps_mm.tile([128, 512], F32, tag="mm")
                    nc.tensor.matmul(ps[:], ow_sb[:], acc[:, ts(hh, 512)], start=True, stop=True)
                    nc.scalar.activation(out=yT[:, ts(hh, 512)], in_=ps[:], func=mybir.ActivationFunctionType.Identity, bias=ob_sb[:], scale=1.0)
                for t in range(8):
                    ps2 = ps_tr.tile([128, 128], F32, tag="tr")
                    nc.tensor.matmul(ps2[:], yT[:, ts(t, 128)], ident[:], is_transpose=True)
                    ob_t = opool.tile([128, 128], F32, tag="ob_t")
                    nc.scalar.copy(out=ob_t[:], in_=ps2[:])
                    nc.sync.dma_start(out=out_d[ts(s * 8 + t, 128), :], in_=ob_t[:])
    nc.compile()
    return nc


def prep_inputs(input, value_w, value_b, offmask_w, offmask_b, out_w, out_b):
    perm = _perm_rows()
    vwT = np.ascontiguousarray(np.asarray(value_w, np.float32).T)
    vb = np.asarray(value_b, np.float32).reshape(C, 1)
    omT = np.ascontiguousarray(np.asarray(offmask_w, np.float32)[perm].T)
    ombr = np.tile(np.asarray(offmask_b, np.float32)[perm][None, :], (128, 1))
    owT = np.ascontiguousarray(np.asarray(out_w, np.float32).T.astype(np.float16))
    ob = np.asarray(out_b, np.float32).reshape(C, 1)
    ident = np.eye(128, dtype=np.float32)
    xf = np.asarray(input, np.float32).reshape(N, D, H * W * C)
    in_maps = []
    for core in range(NCORES):
        n, db = core // 4, core % 4
        d0 = db * DB
        slab = np.zeros((DL, H * W * C), np.float32)
        lo, hi = max(0, d0 - 2), min(D, d0 + DB + 2)
        slab[lo - (d0 - 2):hi - (d0 - 2)] = xf[n, lo:hi]
        in_maps.append(dict(xslab=np.ascontiguousarray(slab.reshape(SLAB, C)),
                            vwT=vwT, vb=vb, omT=omT, ombr=ombr, owT=owT, ob=ob, idin=ident))
    return in_maps


def kernel(**inputs):
    from concourse.bass_utils import run_bass_kernel_spmd
    if "nc" not in _NC_CACHE:
        _NC_CACHE["nc"] = build_nc()
    nc = _NC_CACHE["nc"]
    in_maps = prep_inputs(**inputs)
    res = run_bass_kernel_spmd(nc, in_maps, list(range(NCORES))).results
    out = np.zeros((N, D, H, W, C), np.float32)
    for core in range(NCORES):
        n, db = core // 4, core % 4
        out[n, db * DB:(db + 1) * DB] = res[core]["out"].reshape(DB, H, W, C)
    return out
